# revision 3
# baseline (speedup 1.0000x reference)
"""Fast-feedforward (FFF) tree-routing kernel for Trainium2, 8 NeuronCores.

Problem: nn_FFFLayer (moe_routing). Each of 8192 tokens walks a depth-12
binary tree; at node n: logit = x . w1s[n]; out += GELU(logit) * w2s[n];
next = 2n+1+(logit>0).

Strategy (data-parallel over tokens, 1024/core, chunks of 128):
  Phase 1 (routing): levels 0-8 (511 nodes) get their logits from ONE fused
    PE matmul against a feature-major cache of w1s[0:511]; per-level
    selection/gelu/branch are small DVE/ACT ops. Levels 9-11 gather w1 rows
    per token (indirect DMA) and dot on DVE. Produces, per chunk: scaled
    one-hot masks (node-major, PE-transposed) for levels 0-8, gelu coeffs S
    and node indices IDX for levels 9-11.
  Phase 2 (accumulate): out[t] = sum_d s_d[t] * w2[node_d[t]] computed as PE
    matmuls accumulating in PSUM: levels 0-8 use the scaled masks as lhsT
    against SBUF-resident w2s[0:511]; levels 9-11 use diag(s_d) as lhsT
    against gathered w2 rows.
"""
import numpy as np

import concourse.bass as bass
import concourse.bacc as bacc
import concourse.mybir as mybir
import concourse.tile as tile
from concourse.bass_utils import run_bass_kernel_spmd
from concourse.masks import make_identity

F32 = mybir.dt.float32
I32 = mybir.dt.int32
Alu = mybir.AluOpType
Act = mybir.ActivationFunctionType

TOKENS = 8192
D = 4096
N_NODES = 4095
DEPTH = 12
N_CORES = 8
TPC = TOKENS // N_CORES          # tokens per core
P = 128
CHUNKS = TPC // P                # 8 chunks of 128 tokens
FC = D // P                      # 32 feature chunks
NCACHE_LV = 9                    # levels 0..8 cached (511 nodes)
NCACHE_NODES = 2 ** NCACHE_LV - 1  # 511
CCOLS = 512                      # concat width: [0:127 L0-6][pad][128:256 L7][256:512 L8]
GLV = [9, 10, 11]                # gather levels
GELU_FUNC = Act.Gelu             # test.py sim mode swaps to Relu (CoreSim support)

# column start of each cached level in the 512-wide concat layout
LV_COL = [0, 1, 3, 7, 15, 31, 63, 128, 256]
LV_W = [1, 2, 4, 8, 16, 32, 64, 128, 256]
# w2 row range for each of the 4 transposed mask groups (K=128 each)
W2_GRP_ROWS = [0, 127, 255, 383]


def _build_program():
    nc = bacc.Bacc("TRN2", target_bir_lowering=False, debug=False,
                   enable_asserts=False)
    x_d = nc.dram_tensor("x", [TPC, D], F32, kind="ExternalInput").ap()
    w1s_d = nc.dram_tensor("w1s", [N_NODES, D], F32, kind="ExternalInput").ap()
    w2s_d = nc.dram_tensor("w2s", [N_NODES, D], F32, kind="ExternalInput").ap()
    w1fm_d = nc.dram_tensor("w1fm", [P, FC * CCOLS], F32, kind="ExternalInput").ap()
    iota_d = nc.dram_tensor("iota", [P, 256], F32, kind="ExternalInput").ap()
    out_d = nc.dram_tensor("out", [TPC, D], F32, kind="ExternalOutput").ap()

    with tile.TileContext(nc) as tc:
        with tc.tile_pool(name="persist", bufs=1) as pp:
            ident = pp.tile([P, P], F32)
            make_identity(nc, ident[:])
            iota = pp.tile([P, 256], F32)
            nc.sync.dma_start(out=iota[:], in_=iota_d[:])
            # per-chunk persistent state
            mask_fm = [pp.tile([P, CCOLS], F32, name=f"mfm{c}") for c in range(CHUNKS)]
            S = [pp.tile([P, 16], F32, name=f"S{c}") for c in range(CHUNKS)]
            IDX = [pp.tile([P, 4], I32, name=f"IDX{c}") for c in range(CHUNKS)]

            # ---------------- Phase 1: routing ----------------
            with tc.tile_pool(name="p1", bufs=1) as p1, \
                 tc.tile_pool(name="ps1", bufs=1, space="PSUM") as ps1:
                w1fm_sb = p1.tile([P, FC * CCOLS], F32)
                nc.sync.dma_start(out=w1fm_sb[:], in_=w1fm_d[:])

                for c in range(CHUNKS):
                    x_tm = p1.tile([P, D], F32, tag="x_tm", bufs=2)
                    nc.sync.dma_start(out=x_tm[:], in_=x_d[c * P:(c + 1) * P])

                    # transpose x chunk to feature-major
                    x_fm = p1.tile([P, D], F32, tag="x_fm", bufs=2)
                    for g in range(FC // 4):
                        psT = ps1.tile([P, 512], F32, tag="psT", bufs=2)
                        for j in range(4):
                            fc = g * 4 + j
                            nc.tensor.transpose(
                                out=psT[:, j * P:(j + 1) * P],
                                in_=x_tm[:, fc * P:(fc + 1) * P],
                                identity=ident[:])
                        nc.scalar.copy(x_fm[:, g * 512:(g + 1) * 512], psT[:])

                    # fused logits for levels 0-8 (511 nodes + pad col)
                    psL = ps1.tile([P, CCOLS], F32, tag="psL", bufs=2)
                    for fc in range(FC):
                        nc.tensor.matmul(
                            out=psL[:],
                            lhsT=x_fm[:, fc * P:(fc + 1) * P],
                            rhs=w1fm_sb[:, fc * CCOLS:(fc + 1) * CCOLS],
                            start=(fc == 0), stop=(fc == FC - 1))
                    logits = p1.tile([P, CCOLS], F32, tag="logits", bufs=2)
                    nc.scalar.copy(logits[:], psL[:])

                    masks = p1.tile([P, CCOLS], F32, tag="masks", bufs=2)
                    nc.gpsimd.memset(masks[:, 127:128], 0.0)

                    node = p1.tile([P, 1], F32, tag="node", bufs=2)
                    nc.gpsimd.memset(node[:], 0.0)
                    lg = p1.tile([P, 1], F32, tag="lg", bufs=2)
                    bbit = p1.tile([P, 1], F32, tag="bbit", bufs=2)
                    local = p1.tile([P, 1], F32, tag="local", bufs=2)
                    tmp = p1.tile([P, D // 2], F32, tag="tmp", bufs=2)
                    lg2 = p1.tile([P, 1], F32, tag="lg2", bufs=2)

                    # cached levels 0-8
                    for d in range(NCACHE_LV):
                        st, w = LV_COL[d], LV_W[d]
                        msk = masks[:, st:st + w]
                        if d == 0:
                            nc.gpsimd.memset(masks[:, 0:1], 1.0)
                            nc.vector.tensor_copy(lg[:], logits[:, 0:1])
                        else:
                            nc.vector.tensor_scalar(
                                local[:], node[:], float(-(2 ** d - 1)), None,
                                op0=Alu.add)
                            nc.vector.tensor_scalar(
                                msk, iota[:, 0:w], local[:, 0:1], None,
                                op0=Alu.is_equal)
                            sel = p1.tile([P, 256], F32, tag="sel", bufs=2)
                            nc.vector.tensor_tensor(
                                out=sel[:, 0:w], in0=msk, in1=logits[:, st:st + w],
                                op=Alu.mult)
                            nc.vector.tensor_reduce(
                                out=lg[:], in_=sel[:, 0:w], op=Alu.add,
                                axis=mybir.AxisListType.X)
                        # gelu coeff and scaled mask
                        nc.scalar.activation(S[c][:, d:d + 1], lg[:], GELU_FUNC)
                        nc.vector.tensor_scalar(
                            msk, msk, S[c][:, d:d + 1], None, op0=Alu.mult)
                        # branch: node = 2*node + 1 + (lg > 0)
                        nc.vector.tensor_scalar(
                            bbit[:], lg[:], 0.0, None, op0=Alu.is_gt)
                        nc.vector.tensor_scalar(
                            node[:], node[:], 2.0, 1.0, op0=Alu.mult, op1=Alu.add)
                        nc.vector.tensor_tensor(
                            out=node[:], in0=node[:], in1=bbit[:], op=Alu.add)

                    # gather levels 9-11
                    for j, d in enumerate(GLV):
                        nc.vector.tensor_copy(IDX[c][:, j:j + 1], node[:])
                        w1g = p1.tile([P, D], F32, tag="w1g", bufs=2)
                        nc.gpsimd.indirect_dma_start(
                            out=w1g[:], out_offset=None, in_=w1s_d[:],
                            in_offset=bass.IndirectOffsetOnAxis(
                                ap=IDX[c][:, j:j + 1], axis=0))
                        H = D // 2
                        nc.vector.tensor_tensor(
                            out=tmp[:], in0=x_tm[:, 0:H], in1=w1g[:, 0:H],
                            op=Alu.mult)
                        nc.vector.tensor_reduce(
                            out=lg[:], in_=tmp[:], op=Alu.add,
                            axis=mybir.AxisListType.X)
                        nc.vector.tensor_tensor(
                            out=tmp[:], in0=x_tm[:, H:D], in1=w1g[:, H:D],
                            op=Alu.mult)
                        nc.vector.tensor_reduce(
                            out=lg2[:], in_=tmp[:], op=Alu.add,
                            axis=mybir.AxisListType.X)
                        nc.vector.tensor_tensor(
                            out=lg[:], in0=lg[:], in1=lg2[:], op=Alu.add)
                        nc.scalar.activation(S[c][:, d:d + 1], lg[:], GELU_FUNC)
                        if d != 11:
                            nc.vector.tensor_scalar(
                                bbit[:], lg[:], 0.0, None, op0=Alu.is_gt)
                            nc.vector.tensor_scalar(
                                node[:], node[:], 2.0, 1.0,
                                op0=Alu.mult, op1=Alu.add)
                            nc.vector.tensor_tensor(
                                out=node[:], in0=node[:], in1=bbit[:], op=Alu.add)

                    # transpose scaled masks to node-major for phase 2
                    psM = ps1.tile([P, CCOLS], F32, tag="psM", bufs=2)
                    for g in range(4):
                        nc.tensor.transpose(
                            out=psM[:, g * P:(g + 1) * P],
                            in_=masks[:, g * P:(g + 1) * P],
                            identity=ident[:])
                    nc.scalar.copy(mask_fm[c][:], psM[:])

            # ---------------- Phase 2: accumulate ----------------
            with tc.tile_pool(name="p2", bufs=1) as p2, \
                 tc.tile_pool(name="ps2", bufs=1, space="PSUM") as ps2:
                w2c = []
                for g, r0 in enumerate(W2_GRP_ROWS):
                    t = p2.tile([P, D], F32, name=f"w2c{g}")
                    nc.sync.dma_start(out=t[:], in_=w2s_d[r0:r0 + P])
                    w2c.append(t)

                for c in range(CHUNKS):
                    w2g = []
                    for j, d in enumerate(GLV):
                        t = p2.tile([P, D], F32, tag=f"w2g{j}", bufs=2)
                        nc.gpsimd.indirect_dma_start(
                            out=t[:], out_offset=None, in_=w2s_d[:],
                            in_offset=bass.IndirectOffsetOnAxis(
                                ap=IDX[c][:, j:j + 1], axis=0))
                        w2g.append(t)
                    diags = []
                    for j, d in enumerate(GLV):
                        dg = p2.tile([P, P], F32, tag=f"diag{j}", bufs=2)
                        nc.vector.tensor_scalar(
                            dg[:], ident[:], S[c][:, d:d + 1], None, op0=Alu.mult)
                        diags.append(dg)

                    for h in range(2):
                        psO = ps2.tile([P, D // 2], F32, tag="psO", bufs=2)
                        n_mm = 0
                        pairs = ([(mask_fm[c][:, g * P:(g + 1) * P], w2c[g])
                                  for g in range(4)]
                                 + [(diags[j][:], w2g[j]) for j in range(3)])
                        total = len(pairs) * 4
                        for lhsT, rhs in pairs:
                            for n in range(4):
                                nc.tensor.matmul(
                                    out=psO[:, n * 512:(n + 1) * 512],
                                    lhsT=lhsT,
                                    rhs=rhs[:, h * 2048 + n * 512:
                                            h * 2048 + (n + 1) * 512],
                                    start=(n_mm < 4), stop=(n_mm >= total - 4))
                                n_mm += 1
                        out_sb = p2.tile([P, D // 2], F32, tag="out_sb", bufs=3)
                        nc.scalar.copy(out_sb[:], psO[:])
                        nc.sync.dma_start(
                            out=out_d[c * P:(c + 1) * P,
                                      h * 2048:(h + 1) * 2048],
                            in_=out_sb[:])
    nc.compile()
    return nc


def _host_prep():
    """Host-side constant inputs (iota) — w1fm needs w1s so is built per call."""
    iota = np.tile(np.arange(256, dtype=np.float32), (P, 1))
    return iota


def _make_w1fm(w1s: np.ndarray) -> np.ndarray:
    """Feature-major cache of w1s[0:511] in the 512-col concat layout.

    w1fm[p, fc*512 + col] = w1s[node(col), fc*128 + p]
    col layout: 0..126 -> nodes 0..126, 127 pad(0), 128..255 -> 127..254,
                256..511 -> 255..510
    """
    cols = np.zeros((D, CCOLS), dtype=np.float32)
    cols[:, 0:127] = w1s[0:127].T
    cols[:, 128:256] = w1s[127:255].T
    cols[:, 256:512] = w1s[255:511].T
    # [D, CCOLS] -> [FC, P, CCOLS] -> [P, FC, CCOLS] -> [P, FC*CCOLS]
    return np.ascontiguousarray(
        cols.reshape(FC, P, CCOLS).transpose(1, 0, 2).reshape(P, FC * CCOLS))


_cached_nc = None


def kernel(**inputs) -> np.ndarray:
    global _cached_nc
    x = np.ascontiguousarray(inputs["input"], dtype=np.float32)
    w1s = np.ascontiguousarray(inputs["w1s"], dtype=np.float32)
    w2s = np.ascontiguousarray(inputs["w2s"], dtype=np.float32)
    assert x.shape == (TOKENS, D) and w1s.shape == (N_NODES, D)
    assert int(inputs["depth"]) == DEPTH

    if _cached_nc is None:
        _cached_nc = _build_program()
    nc = _cached_nc

    w1fm = _make_w1fm(w1s)
    iota = _host_prep()
    in_maps = []
    for i in range(N_CORES):
        in_maps.append({
            "x": x[i * TPC:(i + 1) * TPC],
            "w1s": w1s,
            "w2s": w2s,
            "w1fm": w1fm,
            "iota": iota,
        })
    res = run_bass_kernel_spmd(nc, in_maps, core_ids=list(range(N_CORES)))
    return np.concatenate([res.results[i]["out"] for i in range(N_CORES)],
                          axis=0)
